# revision 31
# baseline (speedup 1.0000x reference)
"""Trainium2 Bass kernel for CentroidDistance (poincare pairwise distance).

Math (matches the jax reference):
  c_repr = exp_map_zero(centroid_weight)              # [K, D]
  sqdiff[n,k] = x2[n] + c2[k] - 2 * x.c
  arg[n,k]    = 1 + 2*sqdiff/(u[n]*v[k]),  u = 1-x2, v = 1-c2
  dist        = arccosh(arg)
  node_centroid_dist = dist * mask                    # [1, N, K]
  graph_centroid_dist = sum_n(dist*mask) / sum(mask)  # [1, K]

Device formulation: let Q = arg - 1 >= 0 (for this problem's data
Q is in [0.8, 7]). Q is produced directly in PSUM, laid out
[K=128 partitions, nodes free]:
  Q[k,n] = sum_d (-4*c[k,d]/v[k]) * (x[n,d]/u[n])
         + (2/v[k]) * (x2[n]/u[n]) + (2*c2[k]/v[k]) * (1/u[n])
The D-contraction streams x once in fp16 (halves input traffic;
half-ulp errors random-walk across D=128 so dist stays ~3e-5
accurate) against an fp16 hi/lo split of the tiny centroid table as
the *stationary* operand, plus one fp16 contract=6 matmul for the
rank-2 term. A zero-weight bf16 matmul opens each accumulation group:
it contributes exactly 0 but keeps the PE activity monitor (HAM) at
2.4 GHz -- fp16-only matmul streams do not register as PE activity
and run at half clock. Inputs ride the SP HWDGE ring, outputs the
Activation ring, so the two streams never FIFO-serialize.

dist = arccosh(1 + Q) is then a SINGLE ScalarE pass: we generate a
custom activation-table root where the `sqrt` function's spline
buckets are re-fitted (Taylor at each bucket center, same centers so
the profile/control tables stay valid) to f(x) = arccosh(1 + x), and
point the compiler at it via BASS_ACT_ROOT_JSON_PATH. The compiled
NEFF embeds the patched table. The same instruction's accum_out gives
the per-chunk column sums used for graph_centroid_dist.

Host pads each core's node range with zero columns: Q = 0 there and
the table's zero-input special case returns 0, so pads add 0 to both
outputs. Output leaves the device as [K, nodes] (contiguous per
partition); the host transposes back. Sharding: node dim across 8
cores; centroid table replicated; graph sums reduced on host.
"""

import hashlib
import json
import os
import shutil
import tempfile

import numpy as np
import ml_dtypes

import concourse.mybir as mybir
import concourse.tile as tile
from concourse import bacc
from concourse.bass_utils import run_bass_kernel_spmd

F32 = mybir.dt.float32
F16 = mybir.dt.float16
BF16 = mybir.dt.bfloat16
AF = mybir.ActivationFunctionType
BF16_NP = ml_dtypes.bfloat16

N_CORES = 8
D = 128
K = 128
P = 128
CHUNK = 512            # nodes per matmul (one PSUM bank of fp32)
SC_MAX = 2048          # nodes per super-chunk (4 banks)

_BUILD_CACHE = {}
_ACT_ROOT = None

# test harness hooks: set PROFILE_TMPDIR to capture an NTFF profile on
# the next kernel() call; LAST_EXEC_NS holds the measured exec time.
PROFILE_TMPDIR = None
LAST_EXEC_NS = None


def _make_acosh_act_root():
    """Create an activation-table root where `sqrt`'s spline buckets
    compute arccosh(1 + x) instead. Bucket centers, control tables and
    profile metadata are unchanged, so bucket selection still works;
    only the cubic coefficients (Taylor at each center) are replaced.
    Returns (act_info.json path, short content hash)."""
    global _ACT_ROOT
    if _ACT_ROOT is not None:
        return _ACT_ROOT

    from neuronxcc.driver.Job import Job
    from neuronxcc.driver.jobs.support.FindActInfo import findActInfoFile

    src = findActInfoFile(Job.getPackageDir(), "gen3")
    srcdir = os.path.dirname(src)
    dst = tempfile.mkdtemp(prefix="acosh_act_root_")
    for f in os.listdir(srcdir):
        shutil.copy(os.path.join(srcdir, f), os.path.join(dst, f))

    prof = json.load(open(os.path.join(dst, "sqrt_and_others.json")))
    start = prof["func_to_bkt_start_idx"]["sqrt"]
    bkt_path = os.path.join(dst, "sqrt_and_others_bkt.bin")
    bkt = np.fromfile(bkt_path, dtype=np.float32).reshape(-1, 8).copy()
    assert bkt.shape[0] == prof["bkt_entry_cnt"]

    x0 = bkt[start:, 4].astype(np.float64)
    good = x0 > 1e-30
    xs = np.where(good, x0, 1.0)
    Pq = xs * xs + 2.0 * xs
    g0 = np.arccosh(1.0 + xs)
    g1 = 1.0 / np.sqrt(Pq)
    g2h = -(xs + 1.0) * Pq ** -1.5 / 2.0
    g3s = (-(Pq ** -1.5) + 3.0 * (xs + 1.0) ** 2 * Pq ** -2.5) / 6.0

    FMAX = 3.0e38
    d0 = np.where(good, g0, 0.0)
    d1 = np.where(good & (np.abs(g1) < FMAX), g1, 0.0)
    d2 = np.where(good & (np.abs(g2h) < FMAX), g2h, 0.0)
    d3 = np.where(good & (np.abs(g3s) < FMAX), g3s, 0.0)
    bkt[start:, 0] = d0.astype(np.float32)
    bkt[start:, 1] = d1.astype(np.float32)
    bkt[start:, 2] = d2.astype(np.float32)
    bkt[start:, 3] = d3.astype(np.float32)
    bkt.tofile(bkt_path)

    h = hashlib.sha256(bkt.tobytes()).hexdigest()[:8]
    _ACT_ROOT = (os.path.join(dst, "act_info.json"), int(h, 16) & 0x7FFFFF)
    return _ACT_ROOT


def _build(n512: int):
    key = n512
    if key in _BUILD_CACHE:
        return _BUILD_CACHE[key]

    act_root, act_hash = _make_acosh_act_root()
    os.environ["BASS_ACT_ROOT_JSON_PATH"] = act_root

    nodes = n512 * CHUNK
    # super-chunks of up to 2048 nodes; small chunks first and last so
    # the pipeline ramps and drains faster.
    widths = []
    rem = nodes
    if rem % SC_MAX:
        widths.append(rem % SC_MAX)
        rem -= rem % SC_MAX
    if rem >= 2 * SC_MAX:
        widths = widths + [SC_MAX] * (rem // SC_MAX - 1) + [SC_MAX // 2] * 2
    else:
        widths += [SC_MAX] * (rem // SC_MAX)
    scs = []
    off = 0
    for w in widths:
        scs.append((off, w))
        off += w
    n_sc = len(scs)

    nc = bacc.Bacc("TRN2", target_bir_lowering=False, debug=False,
                   num_devices=N_CORES)
    xh = nc.dram_tensor("xh", [D, nodes], F16, kind="ExternalInput").ap()
    aux = nc.dram_tensor("aux", [6, nodes], F16, kind="ExternalInput").ap()
    cth = nc.dram_tensor("cth", [D, K], F16, kind="ExternalInput").ap()
    ctl = nc.dram_tensor("ctl", [D, K], F16, kind="ExternalInput").ap()
    caux = nc.dram_tensor("caux", [6, K], F16, kind="ExternalInput").ap()
    out = nc.dram_tensor("out", [K, nodes], F32, kind="ExternalOutput").ap()
    gacc = nc.dram_tensor("gacc", [K, n_sc], F32, kind="ExternalOutput").ap()

    with tile.TileContext(nc) as tc:
        with (
            tc.tile_pool(name="consts", bufs=1) as cpool,
            tc.tile_pool(name="xin", bufs=4) as xpool,
            tc.tile_pool(name="ain", bufs=4) as apool,
            tc.tile_pool(name="q", bufs=2, space="PSUM") as qpool,
            tc.tile_pool(name="ot", bufs=3) as opool,
        ):
            cth_sb = cpool.tile([D, K], F16)
            nc.sync.dma_start(cth_sb[:], cth[:])
            ctl_sb = cpool.tile([D, K], F16)
            nc.sync.dma_start(ctl_sb[:], ctl[:])
            caux_sb = cpool.tile([6, K], F16)
            nc.sync.dma_start(caux_sb[:], caux[:])
            ga_sb = cpool.tile([K, n_sc], F32)
            # NEFF-cache-busting marker tied to the activation table
            # contents (the table is not part of the BIR otherwise).
            marker = cpool.tile([1, 1], F32)
            nc.gpsimd.memset(marker[:], float(act_hash))
            # zero bf16 matmul operands: a bf16 matmul accumulating an
            # exact 0 into each PSUM bank keeps the PE activity monitor
            # (HAM) warm -- fp16-only matmul streams run at 1.2 GHz.
            zw = cpool.tile([D, K], BF16)
            nc.gpsimd.memset(zw[:], 0.0)
            zr = cpool.tile([D, CHUNK], BF16)
            nc.gpsimd.memset(zr[:], 0.0)

            for sc, (lo, width) in enumerate(scs):
                nch = width // CHUNK
                in_eng, out_eng = nc.sync, nc.scalar
                xh_t = xpool.tile([D, width], F16, tag="xh")
                in_eng.dma_start(xh_t[:], xh[:, lo:lo + width])
                ain = apool.tile([6, width], F16)
                in_eng.dma_start(ain[:], aux[:, lo:lo + width])

                q = qpool.tile([P, width], F32, tag="q")
                for c in range(nch):
                    nc.tensor.matmul(q[:, c * CHUNK:(c + 1) * CHUNK],
                                     lhsT=zw[:], rhs=zr[:],
                                     start=True, stop=False)
                # weight-major order: few LDWEIGHTS, long moving streams
                for lhs, mov, st, sp in (
                    (cth_sb, xh_t, False, False),
                    (ctl_sb, xh_t, False, False),
                    (caux_sb, ain, False, True),
                ):
                    for c in range(nch):
                        nc.tensor.matmul(
                            q[:, c * CHUNK:(c + 1) * CHUNK],
                            lhsT=lhs[:],
                            rhs=mov[:, c * CHUNK:(c + 1) * CHUNK],
                            start=st, stop=sp)

                ot = opool.tile([P, width], F32, tag="ot")
                # 'Sqrt' table is patched to arccosh(1+x): one ACT pass
                # computes dist and its running column sum.
                nc.scalar.activation(ot[:], q[:], AF.Sqrt,
                                     accum_out=ga_sb[:, sc:sc + 1])
                out_eng.dma_start(out[:, lo:lo + width], ot[:])

            nc.scalar.dma_start(gacc[:], ga_sb[:])

    nc.compile()
    _BUILD_CACHE[key] = nc
    return nc


def _host_prep(node_repr, mask, centroid_weight):
    EPS = 1e-5
    x = np.asarray(node_repr, dtype=np.float64)          # [N, D]
    m = np.asarray(mask, dtype=np.float64).reshape(-1)   # [N]
    cw = np.asarray(centroid_weight, dtype=np.float64)   # [K, D]

    # exp_map_zero on the centroid table
    norm = np.clip(np.linalg.norm(cw, axis=-1, keepdims=True), EPS, None)
    cr = np.tanh(np.clip(norm, None, 15.0)) * cw / norm  # [K, D]

    c2 = np.sum(cr * cr, axis=-1)                        # [K]
    v = 1.0 - c2
    x2 = np.sum(x * x, axis=-1)                          # [N]
    u = 1.0 - x2
    # The reference clips denom=u*v at 1e-5; mirror by flooring factors.
    u = np.maximum(u, 1e-6)
    v = np.maximum(v, 1e-6)

    xs = x / u[:, None]                                  # [N, D] f64
    a = x2 / u                                           # node features
    r = 1.0 / u
    b = 2.0 / v                                          # centroid features
    c = 2.0 * c2 / v

    def hilo16(t):
        hi = t.astype(np.float16)
        lo = (t - hi.astype(np.float64)).astype(np.float16)
        return hi, lo

    a_hi, a_lo = hilo16(a)
    r_hi, r_lo = hilo16(r)
    b_hi, b_lo = hilo16(b)
    c_hi, c_lo = hilo16(c)

    cts = (-4.0 * cr / v[:, None]).T                     # [D, K] f64
    cth = cts.astype(np.float16)
    ctl = (cts - cth.astype(np.float64)).astype(np.float16)
    cth = np.ascontiguousarray(cth)
    ctl = np.ascontiguousarray(ctl)

    caux = np.stack([b_hi, b_lo, b_hi, c_hi, c_lo, c_hi]).astype(np.float16)
    caux = np.ascontiguousarray(caux)                    # [6, K]

    return xs, m, (a_hi, a_lo, r_hi, r_lo), cth, ctl, caux


def kernel(node_repr, mask, centroid_weight):
    n = node_repr.shape[0]
    assert node_repr.shape[1] == D and centroid_weight.shape == (K, D)

    per_core = -(-n // N_CORES)                 # ceil
    n512 = -(-per_core // CHUNK)                # 512-chunks per core
    nodes_c = n512 * CHUNK                      # padded nodes per core

    xs, m, (a_hi, a_lo, r_hi, r_lo), cth, ctl, caux = _host_prep(
        node_repr, mask, centroid_weight)

    nc = _build(n512)

    xsT = xs.T                                  # [D, N] f64 view
    in_maps = []
    for core in range(N_CORES):
        s, e = core * per_core, min((core + 1) * per_core, n)
        cn = e - s
        xst = xsT[:, s:e]
        xh = np.zeros((D, nodes_c), dtype=np.float16)
        xh[:, :cn] = xst.astype(np.float16)
        aux = np.zeros((6, nodes_c), dtype=np.float16)
        aux[0, :cn] = a_hi[s:e]
        aux[1, :cn] = a_hi[s:e]
        aux[2, :cn] = a_lo[s:e]
        aux[3, :cn] = r_hi[s:e]
        aux[4, :cn] = r_hi[s:e]
        aux[5, :cn] = r_lo[s:e]
        in_maps.append({"xh": xh, "aux": aux,
                        "cth": cth, "ctl": ctl, "caux": caux})

    global LAST_EXEC_NS
    kwargs = {}
    if PROFILE_TMPDIR:
        kwargs = {"trace": True, "tmpdir": PROFILE_TMPDIR}
    res = run_bass_kernel_spmd(nc, in_maps, core_ids=list(range(N_CORES)),
                               **kwargs)
    LAST_EXEC_NS = res.exec_time_ns

    parts = []
    g_sum = np.zeros(K, dtype=np.float64)
    for core in range(N_CORES):
        s, e = core * per_core, min((core + 1) * per_core, n)
        dist_t = res.results[core]["out"]               # [K, nodes_c]
        parts.append(np.ascontiguousarray(dist_t[:, :e - s].T))
        g_sum += res.results[core]["gacc"].astype(np.float64).sum(axis=1)

    ncd = np.concatenate(parts, axis=0)[None]           # [1, N, K] f32

    if bool(np.all(m == 1.0)):
        gcd = (g_sum[None] / n).astype(np.float32)      # [1, K]
    else:
        mm = m
        ncd = (ncd * mm[None, :, None]).astype(np.float32)
        gcd = (ncd.astype(np.float64).sum(axis=1) /
               mm.sum()).astype(np.float32)
    return (gcd, ncd)


# revision 34
# speedup vs baseline: 1.1718x; 1.1718x over previous
"""Trainium2 Bass kernel for CentroidDistance (poincare pairwise distance).

Math (matches the jax reference):
  c_repr = exp_map_zero(centroid_weight)              # [K, D]
  sqdiff[n,k] = x2[n] + c2[k] - 2 * x.c
  arg[n,k]    = 1 + 2*sqdiff/(u[n]*v[k]),  u = 1-x2, v = 1-c2
  dist        = arccosh(arg)
  node_centroid_dist = dist * mask                    # [1, N, K]
  graph_centroid_dist = sum_n(dist*mask) / sum(mask)  # [1, K]

Device formulation: let Q = arg - 1 >= 0 (for this problem's data
Q is in [0.8, 7]). Q is produced directly in PSUM, laid out
[K=128 partitions, nodes free]:
  Q[k,n] = sum_d (-4*c[k,d]/v[k]) * (x[n,d]/u[n])
         + (2/v[k]) * (x2[n]/u[n]) + (2*c2[k]/v[k]) * (1/u[n])
The D-contraction streams x once in fp16 (halves input traffic;
half-ulp errors random-walk across D=128 so dist stays ~3e-5
accurate) against an fp16 hi/lo split of the tiny centroid table as
the *stationary* operand, plus one fp16 contract=6 matmul for the
rank-2 term. A zero-weight bf16 matmul opens each accumulation group:
it contributes exactly 0 but keeps the PE activity monitor (HAM) at
2.4 GHz -- fp16-only matmul streams do not register as PE activity
and run at half clock. Inputs ride the SP HWDGE ring, outputs the
Activation ring, so the two streams never FIFO-serialize.

dist = arccosh(1 + Q) is then a SINGLE ScalarE pass: we generate a
custom activation-table root where the `sqrt` function's spline
buckets are re-fitted (Taylor at each bucket center, same centers so
the profile/control tables stay valid) to f(x) = arccosh(1 + x), and
point the compiler at it via BASS_ACT_ROOT_JSON_PATH. The compiled
NEFF embeds the patched table. The same instruction's accum_out gives
the per-chunk column sums used for graph_centroid_dist.

Host pads each core's node range with zero columns: Q = 0 there and
the table's zero-input special case returns 0, so pads add 0 to both
outputs. Output leaves the device as [K, nodes] (contiguous per
partition); the host transposes back. Sharding: node dim across 8
cores; centroid table replicated; graph sums reduced on host.
"""

import hashlib
import json
import os
import shutil
import tempfile

import numpy as np
import ml_dtypes

import concourse.mybir as mybir
import concourse.tile as tile
from concourse import bacc
from concourse.bass_utils import run_bass_kernel_spmd

F32 = mybir.dt.float32
F16 = mybir.dt.float16
BF16 = mybir.dt.bfloat16
AF = mybir.ActivationFunctionType
BF16_NP = ml_dtypes.bfloat16

N_CORES = 8
D = 128
K = 128
P = 128
CHUNK = 512            # nodes per matmul (one PSUM bank of fp32)
SC_MAX = 2048          # nodes per super-chunk (4 banks)

_BUILD_CACHE = {}
_ACT_ROOT = None

# test harness hooks: set PROFILE_TMPDIR to capture an NTFF profile on
# the next kernel() call; LAST_EXEC_NS holds the measured exec time.
PROFILE_TMPDIR = None
LAST_EXEC_NS = None


def _make_acosh_act_root():
    """Create an activation-table root where `sqrt`'s spline buckets
    compute arccosh(1 + x) instead. Bucket centers, control tables and
    profile metadata are unchanged, so bucket selection still works;
    only the cubic coefficients (Taylor at each center) are replaced.
    Returns (act_info.json path, short content hash)."""
    global _ACT_ROOT
    if _ACT_ROOT is not None:
        return _ACT_ROOT

    from neuronxcc.driver.Job import Job
    from neuronxcc.driver.jobs.support.FindActInfo import findActInfoFile

    src = findActInfoFile(Job.getPackageDir(), "gen3")
    srcdir = os.path.dirname(src)
    dst = tempfile.mkdtemp(prefix="acosh_act_root_")
    for f in os.listdir(srcdir):
        shutil.copy(os.path.join(srcdir, f), os.path.join(dst, f))

    prof = json.load(open(os.path.join(dst, "sqrt_and_others.json")))
    start = prof["func_to_bkt_start_idx"]["sqrt"]
    bkt_path = os.path.join(dst, "sqrt_and_others_bkt.bin")
    bkt = np.fromfile(bkt_path, dtype=np.float32).reshape(-1, 8).copy()
    assert bkt.shape[0] == prof["bkt_entry_cnt"]

    x0 = bkt[start:, 4].astype(np.float64)
    good = x0 > 1e-30
    xs = np.where(good, x0, 1.0)
    Pq = xs * xs + 2.0 * xs
    g0 = np.arccosh(1.0 + xs)
    g1 = 1.0 / np.sqrt(Pq)
    g2h = -(xs + 1.0) * Pq ** -1.5 / 2.0
    g3s = (-(Pq ** -1.5) + 3.0 * (xs + 1.0) ** 2 * Pq ** -2.5) / 6.0

    FMAX = 3.0e38
    d0 = np.where(good, g0, 0.0)
    d1 = np.where(good & (np.abs(g1) < FMAX), g1, 0.0)
    d2 = np.where(good & (np.abs(g2h) < FMAX), g2h, 0.0)
    d3 = np.where(good & (np.abs(g3s) < FMAX), g3s, 0.0)
    bkt[start:, 0] = d0.astype(np.float32)
    bkt[start:, 1] = d1.astype(np.float32)
    bkt[start:, 2] = d2.astype(np.float32)
    bkt[start:, 3] = d3.astype(np.float32)
    bkt.tofile(bkt_path)

    h = hashlib.sha256(bkt.tobytes()).hexdigest()[:8]
    _ACT_ROOT = (os.path.join(dst, "act_info.json"), int(h, 16) & 0x7FFFFF)
    return _ACT_ROOT


def _build(n512: int):
    key = n512
    if key in _BUILD_CACHE:
        return _BUILD_CACHE[key]

    act_root, act_hash = _make_acosh_act_root()
    os.environ["BASS_ACT_ROOT_JSON_PATH"] = act_root

    nodes = n512 * CHUNK
    # super-chunks of up to 2048 nodes; small chunks first and last so
    # the pipeline ramps and drains faster.
    widths = []
    rem = nodes
    if rem % SC_MAX:
        widths.append(rem % SC_MAX)
        rem -= rem % SC_MAX
    if rem >= 2 * SC_MAX:
        widths = widths + [SC_MAX] * (rem // SC_MAX - 1) + [SC_MAX // 2] * 2
    else:
        widths += [SC_MAX] * (rem // SC_MAX)
    scs = []
    off = 0
    for w in widths:
        scs.append((off, w))
        off += w
    n_sc = len(scs)

    nc = bacc.Bacc("TRN2", target_bir_lowering=False, debug=False,
                   num_devices=N_CORES)
    xh = nc.dram_tensor("xh", [D, nodes], F16, kind="ExternalInput").ap()
    aux = nc.dram_tensor("aux", [6, nodes], F16, kind="ExternalInput").ap()
    cth = nc.dram_tensor("cth", [D, K], F16, kind="ExternalInput").ap()
    ctl = nc.dram_tensor("ctl", [D, K], F16, kind="ExternalInput").ap()
    caux = nc.dram_tensor("caux", [6, K], F16, kind="ExternalInput").ap()
    out = nc.dram_tensor("out", [K, nodes], F16, kind="ExternalOutput").ap()
    gacc = nc.dram_tensor("gacc", [K, n_sc], F32, kind="ExternalOutput").ap()

    with tile.TileContext(nc) as tc:
        with (
            tc.tile_pool(name="consts", bufs=1) as cpool,
            tc.tile_pool(name="xin", bufs=4) as xpool,
            tc.tile_pool(name="ain", bufs=4) as apool,
            tc.tile_pool(name="q", bufs=2, space="PSUM") as qpool,
            tc.tile_pool(name="ot", bufs=3) as opool,
        ):
            cth_sb = cpool.tile([D, K], F16)
            nc.sync.dma_start(cth_sb[:], cth[:])
            ctl_sb = cpool.tile([D, K], F16)
            nc.sync.dma_start(ctl_sb[:], ctl[:])
            caux_sb = cpool.tile([6, K], F16)
            nc.sync.dma_start(caux_sb[:], caux[:])
            ga_sb = cpool.tile([K, n_sc], F32)
            # NEFF-cache-busting marker tied to the activation table
            # contents (the table is not part of the BIR otherwise).
            marker = cpool.tile([1, 1], F32)
            nc.gpsimd.memset(marker[:], float(act_hash))
            # zero bf16 matmul operands: a bf16 matmul accumulating an
            # exact 0 into each PSUM bank keeps the PE activity monitor
            # (HAM) warm -- fp16-only matmul streams run at 1.2 GHz.
            zw = cpool.tile([D, K], BF16)
            nc.gpsimd.memset(zw[:], 0.0)
            zr = cpool.tile([D, CHUNK], BF16)
            nc.gpsimd.memset(zr[:], 0.0)

            for sc, (lo, width) in enumerate(scs):
                nch = width // CHUNK
                in_eng, out_eng = nc.sync, nc.scalar
                xh_t = xpool.tile([D, width], F16, tag="xh")
                in_eng.dma_start(xh_t[:], xh[:, lo:lo + width])
                ain = apool.tile([6, width], F16)
                in_eng.dma_start(ain[:], aux[:, lo:lo + width])

                q = qpool.tile([P, width], F32, tag="q")
                for c in range(nch):
                    nc.tensor.matmul(q[:, c * CHUNK:(c + 1) * CHUNK],
                                     lhsT=zw[:], rhs=zr[:],
                                     start=True, stop=False)
                # weight-major order: few LDWEIGHTS, long moving streams
                for lhs, mov, st, sp in (
                    (cth_sb, xh_t, False, False),
                    (ctl_sb, xh_t, False, False),
                    (caux_sb, ain, False, True),
                ):
                    for c in range(nch):
                        nc.tensor.matmul(
                            q[:, c * CHUNK:(c + 1) * CHUNK],
                            lhsT=lhs[:],
                            rhs=mov[:, c * CHUNK:(c + 1) * CHUNK],
                            start=st, stop=sp)

                ot = opool.tile([P, width], F16, tag="ot")
                # 'Sqrt' table is patched to arccosh(1+x): one ACT pass
                # computes dist and its running column sum.
                nc.scalar.activation(ot[:], q[:], AF.Sqrt,
                                     accum_out=ga_sb[:, sc:sc + 1])
                out_eng.dma_start(out[:, lo:lo + width], ot[:])

            nc.scalar.dma_start(gacc[:], ga_sb[:])

    nc.compile()
    _BUILD_CACHE[key] = nc
    return nc


def _host_prep(node_repr, mask, centroid_weight):
    EPS = 1e-5
    x = np.asarray(node_repr, dtype=np.float64)          # [N, D]
    m = np.asarray(mask, dtype=np.float64).reshape(-1)   # [N]
    cw = np.asarray(centroid_weight, dtype=np.float64)   # [K, D]

    # exp_map_zero on the centroid table
    norm = np.clip(np.linalg.norm(cw, axis=-1, keepdims=True), EPS, None)
    cr = np.tanh(np.clip(norm, None, 15.0)) * cw / norm  # [K, D]

    c2 = np.sum(cr * cr, axis=-1)                        # [K]
    v = 1.0 - c2
    x2 = np.sum(x * x, axis=-1)                          # [N]
    u = 1.0 - x2
    # The reference clips denom=u*v at 1e-5; mirror by flooring factors.
    u = np.maximum(u, 1e-6)
    v = np.maximum(v, 1e-6)

    xs = x / u[:, None]                                  # [N, D] f64
    a = x2 / u                                           # node features
    r = 1.0 / u
    b = 2.0 / v                                          # centroid features
    c = 2.0 * c2 / v

    def hilo16(t):
        hi = t.astype(np.float16)
        lo = (t - hi.astype(np.float64)).astype(np.float16)
        return hi, lo

    a_hi, a_lo = hilo16(a)
    r_hi, r_lo = hilo16(r)
    b_hi, b_lo = hilo16(b)
    c_hi, c_lo = hilo16(c)

    cts = (-4.0 * cr / v[:, None]).T                     # [D, K] f64
    cth = cts.astype(np.float16)
    ctl = (cts - cth.astype(np.float64)).astype(np.float16)
    cth = np.ascontiguousarray(cth)
    ctl = np.ascontiguousarray(ctl)

    caux = np.stack([b_hi, b_lo, b_hi, c_hi, c_lo, c_hi]).astype(np.float16)
    caux = np.ascontiguousarray(caux)                    # [6, K]

    return xs, m, (a_hi, a_lo, r_hi, r_lo), cth, ctl, caux


def kernel(node_repr, mask, centroid_weight):
    n = node_repr.shape[0]
    assert node_repr.shape[1] == D and centroid_weight.shape == (K, D)

    per_core = -(-n // N_CORES)                 # ceil
    n512 = -(-per_core // CHUNK)                # 512-chunks per core
    nodes_c = n512 * CHUNK                      # padded nodes per core

    xs, m, (a_hi, a_lo, r_hi, r_lo), cth, ctl, caux = _host_prep(
        node_repr, mask, centroid_weight)

    nc = _build(n512)

    xsT = xs.T                                  # [D, N] f64 view
    in_maps = []
    for core in range(N_CORES):
        s, e = core * per_core, min((core + 1) * per_core, n)
        cn = e - s
        xst = xsT[:, s:e]
        xh = np.zeros((D, nodes_c), dtype=np.float16)
        xh[:, :cn] = xst.astype(np.float16)
        aux = np.zeros((6, nodes_c), dtype=np.float16)
        aux[0, :cn] = a_hi[s:e]
        aux[1, :cn] = a_hi[s:e]
        aux[2, :cn] = a_lo[s:e]
        aux[3, :cn] = r_hi[s:e]
        aux[4, :cn] = r_hi[s:e]
        aux[5, :cn] = r_lo[s:e]
        in_maps.append({"xh": xh, "aux": aux,
                        "cth": cth, "ctl": ctl, "caux": caux})

    global LAST_EXEC_NS
    kwargs = {}
    if PROFILE_TMPDIR:
        kwargs = {"trace": True, "tmpdir": PROFILE_TMPDIR}
    res = run_bass_kernel_spmd(nc, in_maps, core_ids=list(range(N_CORES)),
                               **kwargs)
    LAST_EXEC_NS = res.exec_time_ns

    parts = []
    g_sum = np.zeros(K, dtype=np.float64)
    for core in range(N_CORES):
        s, e = core * per_core, min((core + 1) * per_core, n)
        dist_t = res.results[core]["out"]               # [K, nodes_c] f16
        parts.append(np.ascontiguousarray(
            dist_t[:, :e - s].astype(np.float32).T))
        g_sum += res.results[core]["gacc"].astype(np.float64).sum(axis=1)

    ncd = np.concatenate(parts, axis=0)[None]           # [1, N, K] f32

    if bool(np.all(m == 1.0)):
        gcd = (g_sum[None] / n).astype(np.float32)      # [1, K]
    else:
        mm = m
        ncd = (ncd * mm[None, :, None]).astype(np.float32)
        gcd = (ncd.astype(np.float64).sum(axis=1) /
               mm.sum()).astype(np.float32)
    return (gcd, ncd)
